# revision 15
# baseline (speedup 1.0000x reference)
"""Trainium2 Bass kernel for a 2-layer DenseGCN encoder with mean+max readout.

Reference (per graph b; B=256 graphs, N=256 nodes, F=128 features):
    A  = adj with diagonal set to 1.0
    d  = rowsum(A) ** -0.5        (rowsum >= 1: diag=1, offdiag >= 0)
    An = d[:,None] * A * d[None,:]   (S A S, symmetric)
    H1 = An @ X @ W1 + b1
    H2 = An @ H1 @ W2 + b2
    out = concat([mean_n(H2), max_n(H2)]) @ Wr + br

Device mapping, v8. The HOST precomputes the fully normalized An = S A S
(bf16) so the device does no normalization at all -- just the four matmul
stages, three PSUM->SBUF casts, and the two pooling reductions:
    C    = X^T An          (PE, per (g,t) chunks)     -> c_sb   (ACT copy)
    M1   = c_sb^T W1       (PE; = H1, n-partitioned)  -> h1_sb  (POOL copy)
    C2   = h1_sb^T An      (PE; = (An H1)^T)          -> c2_sb  (ACT copy)
    M2T  = W2^T c2_sb      (PE; = H2^T pre-b2, PSUM)
    pooled_s = reduce_sum(M2T), pooled_m = reduce_max(M2T)   (DVE, per graph)
    out = pooled_s^T Wr_s + pooled_m^T Wr_m + 1 br_eff^T  (fp32)   [PE]
b2 and the mean's 1/N are folded into br_eff / Wr_s on the host.

Sharding: data-parallel over the batch dim, 32 graphs per core x 8 cores.
Inputs are cast to bf16 and re-laid out partition-major on the host.
"""

import numpy as np
import ml_dtypes

B, N, F = 256, 256, 128
NCORES = 8
GPC = B // NCORES  # graphs per core
AGSZ = 4  # graphs per adj/x group
NGRP = GPC // AGSZ
ADJ_SCALE = 64.0  # pow2 prescale for fp8 An, folded into W1/W2

_CACHE = {}


def _build_program(with_b1: bool):
    import concourse.bass as bass
    import concourse.mybir as mybir
    import concourse.tile as tile
    from concourse import bacc
    from contextlib import ExitStack

    f32 = mybir.dt.float32
    bf16 = mybir.dt.bfloat16
    f8 = mybir.dt.float8e4
    DR = mybir.MatmulPerfMode.DoubleRow
    ADD = mybir.AluOpType.add
    AX = mybir.AxisListType.X

    nc = bacc.Bacc("TRN2", target_bir_lowering=False, debug=False,
                   num_devices=NCORES)

    # adjin holds the normalized An scaled by ADJ_SCALE, fp8:
    # [128, group, t, g, n]
    adjin = nc.dram_tensor("adjin", [128, NGRP, 2, AGSZ, N], f8,
                           kind="ExternalInput").ap()
    xin = nc.dram_tensor("xin", [128, GPC, 2, F], bf16,
                         kind="ExternalInput").ap()
    cw1 = nc.dram_tensor("cw1", [F, F], bf16, kind="ExternalInput").ap()
    cw2 = nc.dram_tensor("cw2", [F, F], bf16, kind="ExternalInput").ap()
    cwrs = nc.dram_tensor("cwrs", [F, F], f32, kind="ExternalInput").ap()
    cwrm = nc.dram_tensor("cwrm", [F, F], f32, kind="ExternalInput").ap()
    cbr = nc.dram_tensor("cbr", [1, F], f32, kind="ExternalInput").ap()
    cones32 = nc.dram_tensor("cones32", [1, GPC], f32,
                             kind="ExternalInput").ap()
    if with_b1:
        cb1 = nc.dram_tensor("cb1", [128, 2 * AGSZ * F], bf16,
                             kind="ExternalInput").ap()
    out_d = nc.dram_tensor("out", [GPC, F], f32, kind="ExternalOutput").ap()

    with tile.TileContext(nc) as tc, ExitStack() as ctx:
        p_const = ctx.enter_context(tc.tile_pool(name="const", bufs=1))
        p_ag = ctx.enter_context(tc.tile_pool(name="ag", bufs=NGRP))
        p_xg = ctx.enter_context(tc.tile_pool(name="xg", bufs=NGRP))
        p_sb = ctx.enter_context(tc.tile_pool(name="sb", bufs=6))
        p_acc = ctx.enter_context(tc.tile_pool(name="acc", bufs=1))
        p_tiny = ctx.enter_context(tc.tile_pool(name="tiny", bufs=2))
        # PSUM: two shared pools, 2 bufs x 2 banks each = 8 banks total
        ps_a = ctx.enter_context(tc.tile_pool(name="psa", bufs=2, space="PSUM"))
        ps_b = ctx.enter_context(tc.tile_pool(name="psb", bufs=2, space="PSUM"))

        def cload(ap, shape, tag, dt):
            t = p_const.tile(shape, dt, tag=tag, name=tag)
            nc.gpsimd.dma_start(t[:], ap)
            return t

        ag_tiles = [None] * NGRP
        xg_tiles = [None] * NGRP

        # --- DMA: group 0 split fine (per-graph) so C(0) unblocks ASAP;
        # later groups as 2 per-t slices. adj on sync, x on gpsimd. ---
        def load_ag(i):
            t = p_ag.tile([128, 2 * AGSZ * N], f8, tag="ag", name="ag")
            if i == 0:
                # column-major over g so graph 0's two chunks land first
                for g in range(AGSZ):
                    for tt in range(2):
                        nc.sync.dma_start(
                            t[:, (tt * AGSZ + g) * N:(tt * AGSZ + g + 1) * N],
                            adjin[:, i, tt, g])
            else:
                for tt in range(2):
                    nc.sync.dma_start(
                        t[:, tt * AGSZ * N:(tt + 1) * AGSZ * N],
                        adjin[:, i, tt])
            ag_tiles[i] = t

        def load_xg(i, fine=False):
            t = p_xg.tile([128, AGSZ * 2 * F], bf16, tag="xg", name="xg")
            # per-graph (fine) or per-pair slices so each transfer is small
            # enough to land quickly on a single DMA engine
            step = 1 if fine else 2
            for g0 in range(0, AGSZ, step):
                dst = t[:, g0 * 2 * F:(g0 + step) * 2 * F].rearrange(
                    "p (g t f) -> p g t f", g=step, t=2, f=F)
                nc.gpsimd.dma_start(dst, xin[:, i * AGSZ + g0:
                                             i * AGSZ + g0 + step])
            xg_tiles[i] = t

        load_xg(0, fine=True)
        load_ag(0)
        for i in range(1, NGRP):
            load_ag(i)
            load_xg(i)
        w1 = cload(cw1, [F, F], "w1", bf16)
        w2 = cload(cw2, [F, F], "w2", bf16)
        wrs = cload(cwrs, [F, F], "wrs", f32)
        wrm = cload(cwrm, [F, F], "wrm", f32)
        br_row = cload(cbr, [1, F], "br_row", f32)
        ones32 = cload(cones32, [1, GPC], "ones32", f32)
        if with_b1:
            b1bc = cload(cb1, [128, 2 * AGSZ * F], "b1bc", bf16)

        pooled_s = p_acc.tile([F, GPC], f32, tag="pooled_s")
        pooled_m = p_acc.tile([F, GPC], f32, tag="pooled_m")

        state = {}

        def emit_C(j):
            ag, xg = ag_tiles[j], xg_tiles[j]
            c_ps = ps_a.tile([F, AGSZ * N], f32, tag="ca", name="c_ps")
            for g in range(AGSZ):
                for t in range(2):
                    nc.tensor.matmul(
                        c_ps[:, g * N:(g + 1) * N],
                        xg[:, (g * 2 + t) * F:(g * 2 + t + 1) * F],
                        ag[:, (t * AGSZ + g) * N:(t * AGSZ + g + 1) * N],
                        start=(t == 0), stop=(t == 1))
            c_sb = p_sb.tile([F, AGSZ * N], bf16, tag="c_sb", name="c_sb")
            nc.scalar.copy(c_sb[:], c_ps[:])
            state[("c", j)] = c_sb

        def emit_M1(j):
            c_sb = state.pop(("c", j))
            m1_ps = ps_b.tile([128, 2 * AGSZ * F], f32, tag="mb",
                              name="m1_ps")
            for g in range(AGSZ):
                for t in range(2):
                    nc.tensor.matmul(
                        m1_ps[:, (g * 2 + t) * F:(g * 2 + t + 1) * F],
                        c_sb[:, g * N + t * 128:g * N + t * 128 + 128],
                        w1[:], start=True, stop=True)
            h1_sb = p_sb.tile([128, 2 * AGSZ * F], f8, tag="h1_sb",
                              name="h1_sb")
            if with_b1:
                nc.vector.tensor_tensor(out=h1_sb[:], in0=m1_ps[:],
                                        in1=b1bc[:], op=ADD)
            elif j % 3 == 1:
                # balance PSUM-drain copies: ACT paces the pipeline, so
                # route some h1 copies through the (lighter-loaded) DVE
                nc.vector.tensor_copy(h1_sb[:], m1_ps[:])
            else:
                nc.scalar.copy(h1_sb[:], m1_ps[:])
            state[("h1", j)] = h1_sb

        def emit_C2(j):
            # fp8 x fp8 -> DoubleRow: both 128-row chunks in one matmul
            ag = ag_tiles[j]
            agv = ag[:].rearrange("p (t g n) -> p t g n", t=2, g=AGSZ, n=N)
            h1_sb = state.pop(("h1", j))
            h1v = h1_sb[:].rearrange("p (g t f) -> p g t f", g=AGSZ, t=2, f=F)
            c2_ps = ps_a.tile([F, AGSZ * N], f32, tag="ca", name="c2_ps")
            for g in range(AGSZ):
                nc.tensor.matmul(
                    c2_ps[:, g * N:(g + 1) * N],
                    h1v[:, g], agv[:, :, g],
                    start=True, stop=True, perf_mode=DR)
            c2_sb = p_sb.tile([F, AGSZ * N], bf16, tag="c2_sb", name="c2_sb")
            nc.scalar.copy(c2_sb[:], c2_ps[:])
            state[("c2", j)] = c2_sb

        def emit_M2T(j):
            c2_sb = state.pop(("c2", j))
            m2t_ps = ps_b.tile([128, AGSZ * N], f32, tag="mb", name="m2t_ps")
            for h in range(2):
                nc.tensor.matmul(m2t_ps[:, h * 2 * N:(h + 1) * 2 * N],
                                 w2[:], c2_sb[:, h * 2 * N:(h + 1) * 2 * N],
                                 start=True, stop=True)
            g0 = j * AGSZ
            view = m2t_ps[:].rearrange("p (g n) -> p g n", g=AGSZ, n=N)
            nc.vector.reduce_max(pooled_m[:, g0:g0 + AGSZ], view, axis=AX)
            nc.vector.reduce_sum(pooled_s[:, g0:g0 + AGSZ], view, axis=AX)

        def emit_readout(h):
            # out[h] = pooled_s^T Wr_s + pooled_m^T Wr_m + 1 br^T (fp32)
            HG = GPC // 2
            sl = slice(h * HG, (h + 1) * HG)
            out_ps = ps_a.tile([HG, F], f32, tag="ca", name="out_ps")
            nc.tensor.matmul(out_ps[:], pooled_s[:, sl], wrs[:], start=True,
                             stop=False)
            nc.tensor.matmul(out_ps[:], pooled_m[:, sl], wrm[:], start=False,
                             stop=False)
            nc.tensor.matmul(out_ps[:], ones32[:, sl], br_row[:],
                             start=False, stop=True)
            out_sb = p_tiny.tile([HG, F], f32, tag="out_sb", name="out_sb")
            nc.scalar.copy(out_sb[:], out_ps[:])
            nc.sync.dma_start(out_d[sl], out_sb[:])

        # ---- software pipeline over groups (oldest stage first) ----
        for j in range(NGRP + 3):
            if 0 <= j - 3 < NGRP:
                emit_M2T(j - 3)
            if 0 <= j - 2 < NGRP:
                emit_C2(j - 2)
            if 0 <= j - 1 < NGRP:
                emit_M1(j - 1)
            if j < NGRP:
                emit_C(j)
            if j - 3 == NGRP // 2 - 1:
                emit_readout(0)
        emit_readout(1)

    nc.compile()
    return nc


def _prep_consts(W1, b1, W2, b2, Wr, br):
    W1 = np.asarray(W1, np.float32)
    W2 = np.asarray(W2, np.float32)
    Wr = np.asarray(Wr, np.float32)
    b1 = np.asarray(b1, np.float32)
    b2 = np.asarray(b2, np.float32)
    br = np.asarray(br, np.float32)
    bf = ml_dtypes.bfloat16
    consts = {
        # compensate the ADJ_SCALE folded into the fp8 An
        "cw1": np.ascontiguousarray((W1 / ADJ_SCALE).astype(bf)),
        "cw2": np.ascontiguousarray((W2 / ADJ_SCALE).astype(bf)),
        "cwrs": np.ascontiguousarray(Wr[:F] / N),  # fold mean's 1/N
        "cwrm": np.ascontiguousarray(Wr[F:]),
        # fold b2 through Wr into the final bias (both pools shift by b2)
        "cbr": (br + b2 @ Wr[:F] + b2 @ Wr[F:]).reshape(1, F)
            .astype(np.float32),
        "cones32": np.ones((1, GPC), np.float32),
    }
    with_b1 = bool(np.any(b1))
    if with_b1:
        consts["cb1"] = np.tile(b1.reshape(1, F), (128, 2 * AGSZ)).astype(bf)
    return consts, with_b1


def _make_in_maps(x, adj, consts):
    bf = ml_dtypes.bfloat16
    f8 = ml_dtypes.float8_e4m3
    x = np.asarray(x, np.float32).astype(bf)
    adj = np.asarray(adj, np.float32)
    idx = np.arange(N)
    # host-side DenseGCNConv normalization: An = S (A + I - diag) S.
    # Scaled by ADJ_SCALE (pow2, folded into W1/W2) so the fp8 values
    # sit in e4m3's normal range instead of the subnormals.
    a = adj.copy()
    a[:, idx, idx] = 1.0
    d = np.maximum(a.sum(axis=-1), 1.0) ** -0.5  # [B, N]
    an = (d[:, :, None] * (ADJ_SCALE * a) * d[:, None, :]).astype(f8)
    in_maps = []
    for c in range(NCORES):
        # partition-major layouts so DMA descriptors are 4KB-contiguous
        xs = x[c * GPC:(c + 1) * GPC].reshape(GPC, 2, 128, F) \
            .transpose(2, 0, 1, 3)
        asd = an[c * GPC:(c + 1) * GPC]
        # [group, g, t, p, n] -> [p, group, t, g, n]
        asd = asd.reshape(NGRP, AGSZ, 2, 128, N).transpose(3, 0, 2, 1, 4)
        m = {"xin": np.ascontiguousarray(xs),
             "adjin": np.ascontiguousarray(asd)}
        m.update(consts)
        in_maps.append(m)
    return in_maps


def kernel(x, adj, W1, b1, W2, b2, Wr, br):
    from concourse.bass_utils import run_bass_kernel_spmd

    consts, with_b1 = _prep_consts(W1, b1, W2, b2, Wr, br)

    key = ("v11", with_b1)
    if key not in _CACHE:
        _CACHE[key] = _build_program(with_b1)
    nc = _CACHE[key]

    in_maps = _make_in_maps(x, adj, consts)
    res = run_bass_kernel_spmd(nc, in_maps, core_ids=list(range(NCORES)))
    out = np.concatenate([res.results[c]["out"] for c in range(NCORES)],
                         axis=0)
    return out
